# revision 34
# baseline (speedup 1.0000x reference)
"""Trainium2 Bass kernel for nn_MultiHeadClusterAttention (sparse clustered attention).

Sharding: sequence-parallel over n across 8 NeuronCores; centroids replicated;
kmeans centroid sums/counts AllReduced each iteration.

Precision plan (validated numerically on-device and in CPU emulation):
 - kmeans distances: fp16 hi/lo split, 3 terms (xh.ch + xh.cl + xl.ch), all
   accumulated in one f32 PSUM -> ~22-bit effective operand precision at
   1 cyc/row (vs f32's 4 cyc/row). PE fp16 subnormals are IEEE-exact, so the
   lo residuals are stored unscaled.
 - kmeans scatter: one-hot (exact fp16) x fp16 hi/lo rows -> ~16-bit sums.
 - attention: fp16 q/k/v/energies (~11.8-bit, matches the old f32r quality),
   k gets the centroid lo term as well; bf16 exp/attV.
 - softmax: no max-subtraction (energies shifted by -39); colsum via ones row
   in the V stationary; 1/Z via ACT Exp(-Ln(Z)); 1/sqrt(E) folded into V.
biases: bq/bk on-device per-partition; bv/bo folded on host.

Q projection runs inside the 10 AllReduce gaps (PE otherwise idle there), and
Q stays in SBUF (no DRAM staging round-trip).
"""
import numpy as np

import concourse.bacc as bacc
import concourse.mybir as mybir
import concourse.tile as tile
from concourse.bass_utils import run_bass_kernel_spmd

NCORES = 8
N, E, NH = 32768, 512, 8
KC = N // 100            # 327
ITERS = 10
NL = N // NCORES         # 4096
P = 128
NCH = NL // P            # 32
NB = [(0, 128), (128, 128), (256, 71)]
NS = NL // 512           # 8 s-blocks
INVSQRT_E = 1.0 / float(np.sqrt(np.float32(E)))

f32 = mybir.dt.float32
bf16 = mybir.dt.bfloat16
fp16 = mybir.dt.float16
AF = mybir.ActivationFunctionType
ALU = mybir.AluOpType
AX = mybir.AxisListType

# jnp.linspace(0, N-1, KC).astype(int32) on CPU jax (harness reference backend)
INIT_IDX = np.array([0, 100, 201, 301, 402, 502, 603, 703, 804, 904, 1005, 1105, 1206, 1306, 1407, 1507, 1608, 1708, 1809, 1909, 2010, 2110, 2211, 2311, 2412, 2512, 2613, 2713, 2814, 2914, 3015, 3115, 3216, 3316, 3417, 3517, 3618, 3718, 3819, 3919, 4020, 4121, 4221, 4322, 4422, 4523, 4623, 4724, 4824, 4925, 5025, 5126, 5226, 5327, 5427, 5528, 5628, 5729, 5829, 5930, 6030, 6131, 6231, 6332, 6432, 6533, 6633, 6734, 6834, 6935, 7035, 7136, 7236, 7337, 7437, 7538, 7638, 7739, 7839, 7940, 8040, 8141, 8242, 8342, 8443, 8543, 8644, 8744, 8845, 8945, 9046, 9146, 9247, 9347, 9448, 9548, 9649, 9749, 9850, 9950, 10051, 10151, 10252, 10352, 10453, 10553, 10654, 10754, 10855, 10955, 11056, 11156, 11257, 11357, 11458, 11558, 11659, 11759, 11860, 11960, 12061, 12161, 12262, 12363, 12463, 12564, 12664, 12765, 12865, 12966, 13066, 13167, 13267, 13368, 13468, 13569, 13669, 13770, 13870, 13971, 14071, 14172, 14272, 14373, 14473, 14574, 14674, 14775, 14875, 14976, 15076, 15177, 15277, 15378, 15478, 15579, 15679, 15780, 15880, 15981, 16081, 16182, 16282, 16383, 16484, 16584, 16685, 16785, 16886, 16986, 17087, 17187, 17288, 17388, 17489, 17589, 17690, 17790, 17891, 17991, 18092, 18192, 18293, 18393, 18494, 18594, 18695, 18795, 18896, 18996, 19097, 19197, 19298, 19398, 19499, 19599, 19700, 19800, 19901, 20001, 20102, 20202, 20303, 20403, 20504, 20605, 20705, 20806, 20906, 21007, 21107, 21208, 21308, 21409, 21509, 21610, 21710, 21811, 21911, 22012, 22112, 22213, 22313, 22414, 22514, 22615, 22715, 22816, 22916, 23017, 23117, 23218, 23318, 23419, 23519, 23620, 23720, 23821, 23921, 24022, 24122, 24223, 24323, 24424, 24524, 24625, 24726, 24826, 24927, 25027, 25128, 25228, 25329, 25429, 25530, 25630, 25731, 25831, 25932, 26032, 26133, 26233, 26334, 26434, 26535, 26635, 26736, 26836, 26937, 27037, 27138, 27238, 27339, 27439, 27540, 27640, 27741, 27841, 27942, 28042, 28143, 28243, 28344, 28444, 28545, 28645, 28746, 28847, 28947, 29048, 29148, 29249, 29349, 29450, 29550, 29651, 29751, 29852, 29952, 30053, 30153, 30254, 30354, 30455, 30555, 30656, 30756, 30857, 30957, 31058, 31158, 31259, 31359, 31460, 31560, 31661, 31761, 31862, 31962, 32063, 32163, 32264, 32364, 32465, 32565, 32666, 32767], dtype=np.int32)


def _build(n_iters=ITERS, debug=False, fake_ar=False, phase="all", coll="rsag"):
    nc = bacc.Bacc("TRN2", target_bir_lowering=False, debug=False, num_devices=NCORES)
    xd = nc.dram_tensor("x", [NL, E], f32, kind="ExternalInput")
    cd = nc.dram_tensor("c0", [KC, E], f32, kind="ExternalInput")
    wd = {w: nc.dram_tensor(w, [E, E], f32, kind="ExternalInput")
          for w in ("wq", "wk", "wv", "wo")}
    bqd = nc.dram_tensor("bq", [E], f32, kind="ExternalInput")
    bkd = nc.dram_tensor("bk", [E], f32, kind="ExternalInput")
    outd = nc.dram_tensor("out", [NL, E], f32, kind="ExternalOutput")
    if debug:
        dbg_c = nc.dram_tensor("dbg_c", [KC, E], f32, kind="ExternalOutput")
        dbg_q = nc.dram_tensor("dbg_q", [P, NL], f32, kind="ExternalOutput")
        dbg_kt = nc.dram_tensor("dbg_kt", [P, 4 * KC], f32, kind="ExternalOutput")
        dbg_sums = nc.dram_tensor("dbg_sums", [KC, 513], f32, kind="ExternalOutput")

    with tile.TileContext(nc) as tc:
        with (
            tc.tile_pool(name="per", bufs=1) as per,
            tc.tile_pool(name="dram", bufs=1, space="DRAM") as dram,
        ):
            # ---------- persistent SBUF (whole kernel) ----------
            q = per.tile([P, 4 * NL], fp16, tag="q")           # (r, s): 4096r+512s
            cht = per.tile([P, 4 * KC], fp16, tag="cht")       # C^T hi, block r at KC*r
            clt = per.tile([P, 4 * KC], fp16, tag="clt")       # C^T lo
            crow = per.tile([P, 3 * E], f32, tag="crow")       # C rows, block b at E*b
            c2t = per.tile([P, KC], f32, tag="c2t")
            wqt = per.tile([P, 4 * E], fp16, tag="wqt")        # (k, r): 512k+128r
            ident = per.tile([P, P], f32, tag="ident")
            ones_h = per.tile([P, 1], fp16, tag="onesh")
            bq_c = per.tile([P, 4], f32, tag="bqc")
            bk_c = per.tile([P, 4], f32, tag="bkc")
            nbias = per.tile([P, 1], f32, tag="nbias")
            arin = dram.tile([KC, 513], f32, tag="arin")
            arout = dram.tile([KC, 513], f32, tag="arout")
            KCP, SL = 328, 41   # padded centroid count, per-core RS slice
            arin2 = dram.tile([KCP, 1025], f32, tag="arin2")
            rsout = dram.tile([SL, 1025], f32, tag="rsout")
            agin = dram.tile([SL, E], f32, tag="agin")
            agout = dram.tile([KCP, E], f32, tag="agout")

            it32 = per.tile([P, P], mybir.dt.int32, tag="it32")
            nc.gpsimd.iota(it32[:], [[1, P]], base=0, channel_multiplier=-1)
            nc.vector.tensor_scalar(ident[:], it32[:], 0, None, ALU.is_equal)
            nc.gpsimd.memset(ones_h[:], 1.0)
            nc.gpsimd.memset(nbias[:], -39.0)
            for r in range(4):
                nc.sync.dma_start(bq_c[:, r:r + 1], bqd[P * r:P * (r + 1)])
                nc.sync.dma_start(bk_c[:, r:r + 1], bkd[P * r:P * (r + 1)])


            # helpers: per-block ct rebuild (+ c2) from crow — block-major so
            # block b's transposes run while block b+1 still updates on DVE
            def emit_ct_block(sbk, psK, b, b0, nb, c2r):
                for r in range(4):
                    tp = psK.tile([P, 512], f32, tag="dps", name="tp")
                    nc.tensor.transpose(tp[:, :nb],
                                        crow[:nb, E * b + P * r:E * b + P * (r + 1)],
                                        ident[:nb, :nb])
                    nc.scalar.activation(cht[:, KC * r + b0:KC * r + b0 + nb],
                                         tp[:, :nb], AF.Copy)
                    nc.vector.tensor_tensor(clt[:, KC * r + b0:KC * r + b0 + nb],
                                            tp[:, :nb],
                                            cht[:, KC * r + b0:KC * r + b0 + nb],
                                            ALU.subtract)
                if c2r is None:
                    return
                sq = sbk.tile([P, E], f32, tag="t1", name="sq")
                nc.vector.tensor_tensor(sq[:nb], crow[:nb, E * b:E * (b + 1)],
                                        crow[:nb, E * b:E * (b + 1)], ALU.mult)
                c2col = sbk.tile([P, 1], f32, tag="c2col", name="c2col")
                nc.vector.tensor_reduce(c2col[:nb], sq[:nb], AX.X, ALU.add)
                c2ps = psK.tile([P, 512], f32, tag="dps", name="c2ps")
                nc.tensor.transpose(c2ps[:1, :nb], c2col[:nb], ident[:nb, :nb])
                nc.scalar.activation(c2r[:, b0:b0 + nb], c2ps[:1, :nb], AF.Copy)

            def emit_ct_from_crow(sbk, psK, with_c2=True):
                c2r = sbk.tile([1, KC], f32, tag="c2r", name="c2r") if with_c2 else None
                for b, (b0, nb) in enumerate(NB):
                    emit_ct_block(sbk, psK, b, b0, nb, c2r)
                if with_c2:
                    nc.gpsimd.partition_broadcast(c2t[:], c2r[:])

            # ======== phase K: prep + kmeans (+ Qproj in AR gaps) ========
            with (
                tc.tile_pool(name="xbig", bufs=1) as xb,
                tc.tile_pool(name="sbk", bufs=1) as sbk,
                tc.tile_pool(name="d2p", bufs=3) as d2p,
                tc.tile_pool(name="psK", bufs=2, space="PSUM") as psK,
                tc.tile_pool(name="psS", bufs=1, space="PSUM") as psS,
            ):
                xht = xb.tile([P, 4 * NL], fp16, tag="xht")    # x^T hi (k,ch): 4096k+128ch
                xlt = xb.tile([P, 4 * NL], fp16, tag="xlt")    # x^T lo
                xhr = xb.tile([P, NCH * E], fp16, tag="xhr")   # x rows hi, ch at 512ch
                xlr = xb.tile([P, NCH * E], fp16, tag="xlr")   # x rows lo

                def emit_xprep(ch):
                    # x chunk load -> fp16 hi/lo rows + transposed hi/lo
                    xc = sbk.tile([P, E], f32, tag="xc", bufs=3)
                    nc.sync.dma_start(xc[:], xd[P * ch:P * (ch + 1), :])
                    # spread conversions across engines: ACT does the fp16
                    # rounding copy, gpsimd the row residual, DVE (below) the
                    # transposed residual — keeps DVE off the critical path
                    nc.scalar.activation(xhr[:, E * ch:E * (ch + 1)], xc[:], AF.Copy)
                    nc.vector.tensor_tensor(xlr[:, E * ch:E * (ch + 1)], xc[:],
                                            xhr[:, E * ch:E * (ch + 1)], ALU.subtract)
                    tp = psK.tile([P, 512], f32, tag="dps", name="tpbig")
                    for k in range(4):
                        nc.tensor.transpose(tp[:, P * k:P * (k + 1)],
                                            xc[:, P * k:P * (k + 1)], ident[:])
                    for k in range(4):
                        sl = slice(NL * k + P * ch, NL * k + P * (ch + 1))
                        nc.scalar.activation(xht[:, sl], tp[:, P * k:P * (k + 1)], AF.Copy)
                        nc.vector.tensor_tensor(xlt[:, sl], tp[:, P * k:P * (k + 1)],
                                                xht[:, sl], ALU.subtract)

                # Wq load + transpose -> fp16 (needed for gap-filled Qproj)
                for r in range(4):
                    worig = sbk.tile([P, E], f32, tag="worig", name="worig")
                    nc.sync.dma_start(worig[:], wd["wq"][P * r:P * (r + 1), :])
                    for k in range(4):
                        tp = psK.tile([P, 512], f32, tag="dps", name="tp")
                        nc.tensor.transpose(tp[:, :P], worig[:, P * k:P * (k + 1)], ident[:])
                        nc.scalar.activation(wqt[:, E * k + P * r:E * k + P * (r + 1)],
                                             tp[:, :P], AF.Copy)

                # c0 -> crow -> cht/clt/c2t; zero the RS pad row
                zrow = sbk.tile([P, 3 * 513], f32, tag="stage", name="zrow")
                nc.gpsimd.memset(zrow[:1, 0:1025], 0.0)
                nc.sync.dma_start(arin2[KCP - 1:KCP, :], zrow[:1, 0:1025])
                for b, (b0, nb) in enumerate(NB):
                    nc.sync.dma_start(crow[:nb, E * b:E * b + E], cd[b0:b0 + nb, :])
                emit_ct_from_crow(sbk, psK)

                for it in range(n_iters):
                    sps_l = [psS.tile([P, 512], f32, tag=f"s{b}", name=f"sps{b}")
                             for b in range(3)]
                    cps = [psS.tile([P, 1], f32, tag=f"c{b}", name=f"cps{b}")
                           for b in range(3)]
                    ohs = []
                    LAG = 3
                    if it == 0:
                        for ch in range(LAG):
                            emit_xprep(ch)
                    for ch in range(NCH):
                        if it == 0 and ch + LAG < NCH:
                            emit_xprep(ch + LAG)
                        # distances: 12 fp16 matmuls into one f32 PSUM
                        dps = psK.tile([P, 512], f32, tag="dps", name="dps")
                        for r in range(4):
                            xsl = slice(NL * r + P * ch, NL * r + P * (ch + 1))
                            csl = slice(KC * r, KC * r + KC)
                            nc.tensor.matmul(dps[:, :KC], xht[:, xsl], cht[:, csl],
                                             start=(r == 0), stop=False)
                            nc.tensor.matmul(dps[:, :KC], xht[:, xsl], clt[:, csl],
                                             start=False, stop=False)
                            nc.tensor.matmul(dps[:, :KC], xlt[:, xsl], cht[:, csl],
                                             start=False, stop=(r == 3))
                        d2 = d2p.tile([P, KC], f32, tag="d2")
                        nc.vector.scalar_tensor_tensor(d2[:], dps[:, :KC], -2.0, c2t[:],
                                                       ALU.mult, ALU.add)
                        mn = d2p.tile([P, 1], f32, tag="mn")
                        nc.vector.tensor_reduce(mn[:], d2[:], AX.X, ALU.min)
                        oh = d2p.tile([P, KC], fp16, tag="oh")
                        nc.vector.tensor_scalar(oh[:], d2[:], mn[:], None, ALU.is_le)
                        ohs.append(oh)
                        # scatter for the previous chunk (keeps PE busy while
                        # DVE computes this chunk's argmin)
                        if ch >= 2:
                            emit_scatter(nc, sps_l, cps, ohs[ch - 2], xhr, xlr,
                                         ch - 2, ones_h)
                    emit_scatter(nc, sps_l, cps, ohs[NCH - 2], xhr, xlr,
                                 NCH - 2, ones_h)
                    emit_scatter(nc, sps_l, cps, ohs[NCH - 1], xhr, xlr,
                                 NCH - 1, ones_h)

                    # stage sums/cnt to DRAM
                    stage = sbk.tile([P, 3 * 513], f32, tag="stage", name="stage")
                    for b, (b0, nb) in enumerate(NB):
                        nc.scalar.activation(stage[:nb, 513 * b:513 * b + 512],
                                             sps_l[b][:nb, :], AF.Copy)
                        nc.scalar.activation(stage[:nb, 513 * b + 512:513 * (b + 1)],
                                             cps[b][:nb, :], AF.Copy)
                    if debug and it == 0:
                        for b, (b0, nb) in enumerate(NB):
                            nc.sync.dma_start(dbg_sums[b0:b0 + nb, :],
                                              stage[:nb, 513 * b:513 * (b + 1)])
                    if coll == "rsag":
                        # ReduceScatter(sums|cnt|crow) -> per-core slice update
                        # -> AllGather(updated centroids). crow is replicated,
                        # so sum/8 recovers it exactly (x0.125 is exact).
                        for b, (b0, nb) in enumerate(NB):
                            nc.sync.dma_start(arin2[b0:b0 + nb, 0:513],
                                              stage[:nb, 513 * b:513 * (b + 1)])
                            nc.sync.dma_start(arin2[b0:b0 + nb, 513:1025],
                                              crow[:nb, E * b:E * (b + 1)])
                        if fake_ar:
                            nc.sync.dma_start(rsout[:], arin2[0:SL, :])
                        else:
                            nc.gpsimd.collective_compute(
                                "ReduceScatter", ALU.add,
                                replica_groups=[list(range(NCORES))],
                                ins=[arin2.opt()], outs=[rsout.opt()],
                            )
                    else:
                        for b, (b0, nb) in enumerate(NB):
                            nc.sync.dma_start(arin[b0:b0 + nb, :],
                                              stage[:nb, 513 * b:513 * (b + 1)])
                        if fake_ar:
                            nc.sync.dma_start(arout[:], arin[:])
                        else:
                            nc.gpsimd.collective_compute(
                                "AllReduce", ALU.add,
                                replica_groups=[list(range(NCORES))],
                                ins=[arin.opt()], outs=[arout.opt()],
                            )

                    # Q projection for s-block `it` — fills the AR gap (PE idle)
                    if it < NS and phase == "all":
                        for r in range(4):
                            qps = psK.tile([P, 512], f32, tag="dps", name="qps")
                            for k in range(4):
                                nc.tensor.matmul(
                                    qps[:],
                                    wqt[:, E * k + P * r:E * k + P * (r + 1)],
                                    xht[:, NL * k + 512 * it:NL * k + 512 * (it + 1)],
                                    start=(k == 0), stop=(k == 3))
                            nc.vector.tensor_scalar(q[:, NL * r + 512 * it:NL * r + 512 * (it + 1)],
                                                    qps[:], bq_c[:, r:r + 1], None, ALU.add)

                    c2r = (sbk.tile([1, KC], f32, tag="c2r", name="c2r")
                           if it < n_iters - 1 else None)
                    if coll == "rsag":
                        # local update of this core's 41-centroid slice
                        rsb = sbk.tile([P, 3 * 513], f32, tag="stage", name="rsb")
                        nc.sync.dma_start(rsb[:SL, 0:1025], rsout[:, :])
                        cnt = rsb[:SL, 512:513]
                        cm = sbk.tile([P, 1], f32, tag="cm")
                        nc.vector.tensor_scalar(cm[:SL], cnt, 1.0, None, ALU.max)
                        rec = sbk.tile([P, 1], f32, tag="rec")
                        nc.vector.reciprocal(rec[:SL], cm[:SL])
                        mpos = sbk.tile([P, 1], f32, tag="mpos")
                        nc.vector.tensor_scalar(mpos[:SL], cnt, 0.0, None, ALU.is_gt)
                        mneg = sbk.tile([P, 1], f32, tag="mneg")
                        nc.vector.tensor_scalar(mneg[:SL], cnt, 0.0, None, ALU.is_le)
                        t1 = sbk.tile([P, E], f32, tag="t1")
                        nc.vector.tensor_scalar(t1[:SL], rsb[:SL, 0:512],
                                                rec[:SL], mpos[:SL], ALU.mult, op1=ALU.mult)
                        t2 = sbk.tile([P, E], f32, tag="t2")
                        # recover replicated crow slice: (8*crow)*0.125, exact
                        nc.vector.tensor_scalar(t2[:SL], rsb[:SL, 513:1025],
                                                0.125, mneg[:SL], ALU.mult, op1=ALU.mult)
                        nc.vector.tensor_tensor(t1[:SL], t1[:SL], t2[:SL], ALU.add)
                        nc.sync.dma_start(agin[:, :], t1[:SL, :])
                        if fake_ar:
                            for g in range(NCORES):
                                nc.sync.dma_start(agout[SL * g:SL * (g + 1), :],
                                                  agin[:, :])
                        else:
                            nc.gpsimd.collective_compute(
                                "AllGather", ALU.bypass,
                                replica_groups=[list(range(NCORES))],
                                ins=[agin.opt()], outs=[agout.opt()],
                            )
                        for b, (b0, nb) in enumerate(NB):
                            nc.sync.dma_start(crow[:nb, E * b:E * (b + 1)],
                                              agout[b0:b0 + nb, :])
                        for b, (b0, nb) in enumerate(NB):
                            emit_ct_block(sbk, psK, b, b0, nb, c2r)
                    else:
                        red = sbk.tile([P, 3 * 513], f32, tag="stage", name="red")
                        for b, (b0, nb) in enumerate(NB):
                            nc.sync.dma_start(red[:nb, 513 * b:513 * (b + 1)],
                                              arout[b0:b0 + nb, :])
                        for b, (b0, nb) in enumerate(NB):
                            cnt = red[:nb, 513 * b + 512:513 * (b + 1)]
                            cm = sbk.tile([P, 1], f32, tag="cm")
                            nc.vector.tensor_scalar(cm[:nb], cnt, 1.0, None, ALU.max)
                            rec = sbk.tile([P, 1], f32, tag="rec")
                            nc.vector.reciprocal(rec[:nb], cm[:nb])
                            mpos = sbk.tile([P, 1], f32, tag="mpos")
                            nc.vector.tensor_scalar(mpos[:nb], cnt, 0.0, None, ALU.is_gt)
                            mneg = sbk.tile([P, 1], f32, tag="mneg")
                            nc.vector.tensor_scalar(mneg[:nb], cnt, 0.0, None, ALU.is_le)
                            t1 = sbk.tile([P, E], f32, tag="t1")
                            nc.vector.tensor_scalar(t1[:nb], red[:nb, 513 * b:513 * b + 512],
                                                    rec[:nb], mpos[:nb], ALU.mult, op1=ALU.mult)
                            t2 = sbk.tile([P, E], f32, tag="t2")
                            nc.vector.tensor_scalar(t2[:nb], crow[:nb, E * b:E * (b + 1)],
                                                    mneg[:nb], None, ALU.mult)
                            nc.vector.tensor_tensor(crow[:nb, E * b:E * (b + 1)],
                                                    t1[:nb], t2[:nb], ALU.add)
                            emit_ct_block(sbk, psK, b, b0, nb, c2r)
                    if c2r is not None:
                        nc.gpsimd.partition_broadcast(c2t[:], c2r[:])

                if n_iters == 0:
                    for ch in range(NCH):
                        emit_xprep(ch)
                # leftover Q projection s-blocks (if ITERS < NS, normally none)
                if phase == "all":
                    for s in range(min(n_iters, NS), NS):
                        for r in range(4):
                            qps = psK.tile([P, 512], f32, tag="dps", name="qps")
                            for k in range(4):
                                nc.tensor.matmul(
                                    qps[:],
                                    wqt[:, E * k + P * r:E * k + P * (r + 1)],
                                    xht[:, NL * k + 512 * s:NL * k + 512 * (s + 1)],
                                    start=(k == 0), stop=(k == 3))
                            nc.vector.tensor_scalar(q[:, NL * r + 512 * s:NL * r + 512 * (s + 1)],
                                                    qps[:], bq_c[:, r:r + 1], None, ALU.add)

            if debug:
                for b, (b0, nb) in enumerate(NB):
                    nc.sync.dma_start(dbg_c[b0:b0 + nb, :], crow[:nb, E * b:E * (b + 1)])

            # ======== phase A: attention ========
            if phase == "all":
                with (
                    tc.tile_pool(name="sw3", bufs=1) as sw3,
                    tc.tile_pool(name="sba", bufs=1) as sba,
                    tc.tile_pool(name="expp", bufs=20) as expp,
                    tc.tile_pool(name="oatp", bufs=3) as oatp,
                    tc.tile_pool(name="alp", bufs=4) as alp,
                    tc.tile_pool(name="outp", bufs=3) as outp,
                    tc.tile_pool(name="psA", bufs=2, space="PSUM") as psA,
                ):
                    wt = {}
                    for w in ("wk", "wv", "wo"):
                        wt[w] = sw3.tile([P, 4 * E], fp16, tag=f"{w}t", name=f"{w}t")
                        for r in range(4):
                            worig = sba.tile([P, E], f32, tag="worig", name="worig")
                            nc.sync.dma_start(worig[:], wd[w][P * r:P * (r + 1), :])
                            for k in range(4):
                                tp = psA.tile([P, 512], f32, tag="e", name="tp", bufs=4)
                                nc.tensor.transpose(tp[:, :P], worig[:, P * k:P * (k + 1)],
                                                    ident[:])
                                nc.scalar.activation(wt[w][:, E * k + P * r:E * k + P * (r + 1)],
                                                     tp[:, :P], AF.Copy)
                    # KT = Wk @ C^T + bk (fp16; centroid hi+lo terms)
                    kt = sba.tile([P, 4 * KC], fp16, tag="kt", name="kt")
                    for r in range(4):
                        kps = psA.tile([P, 512], f32, tag="e", name="kps", bufs=4)
                        for k in range(4):
                            wsl = wt["wk"][:, E * k + P * r:E * k + P * (r + 1)]
                            nc.tensor.matmul(kps[:, :KC], wsl, cht[:, KC * k:KC * (k + 1)],
                                             start=(k == 0), stop=False)
                            nc.tensor.matmul(kps[:, :KC], wsl, clt[:, KC * k:KC * (k + 1)],
                                             start=False, stop=(k == 3))
                        nc.vector.tensor_scalar(kt[:, KC * r:KC * (r + 1)], kps[:, :KC],
                                                bk_c[:, r:r + 1], None, ALU.add)
                    # V -> Vaug bf16 (x 1/sqrt(E); ones col per head)
                    ones_b = sba.tile([P, 1], bf16, tag="onesb", name="onesb")
                    nc.gpsimd.memset(ones_b[:], 1.0)
                    vaug = []
                    for b, (b0, nb) in enumerate(NB):
                        va = sba.tile([P, 65 * NH], bf16, tag=f"vaug{b}", name=f"vaug{b}")
                        vaug.append(va)
                        vps = psA.tile([P, 512], f32, tag="u", name="vps", bufs=2)
                        for k in range(4):
                            nc.tensor.matmul(vps[:nb, :],
                                             cht[:, KC * k + b0:KC * k + b0 + nb],
                                             wt["wv"][:, E * k:E * (k + 1)],
                                             start=(k == 0), stop=(k == 3))
                        for h in range(NH):
                            nc.scalar.activation(va[:nb, 65 * h:65 * h + 64],
                                                 vps[:nb, 64 * h:64 * (h + 1)],
                                                 AF.Copy, scale=INVSQRT_E)
                            nc.vector.tensor_copy(va[:nb, 65 * h + 64:65 * (h + 1)],
                                                  ones_b[:nb])

                    if debug:
                        ktd = sba.tile([P, 4 * KC], f32, tag="ktd", name="ktd")
                        nc.vector.tensor_copy(ktd[:], kt[:])
                        nc.sync.dma_start(dbg_kt[:], ktd[:])
                        qd = sba.tile([P, NL], f32, tag="qd", name="qd")
                        nc.vector.tensor_copy(qd[:], q[:, 0:NL])
                        nc.sync.dma_start(dbg_q[:], qd[:])

                    # software-pipelined s-loop: energies+exp for unit u,
                    # attV/alpha/oat for unit u-1, outproj once a block's 4
                    # oats are emitted. Keeps PE busy while ACT runs exp.
                    units = [(s, i) for s in range(NS) for i in range(4)]
                    oats_by_s = [[None] * 4 for _ in range(NS)]

                    def emit_energy(s, i):
                        qsl = q[:, NL * i + 512 * s:NL * i + 512 * (s + 1)]
                        ex = {0: [], 1: []}
                        for b, (b0, nb) in enumerate(NB):
                            eA = psA.tile([P, 512], f32, tag="e", name="eA", bufs=4)
                            nc.tensor.matmul(eA[:nb, :],
                                             kt[0:64, KC * i + b0:KC * i + b0 + nb],
                                             qsl[0:64, :], start=True, stop=True)
                            eB = psA.tile([P, 512], f32, tag="e", name="eB", bufs=4)
                            nc.tensor.matmul(eB[:nb, :],
                                             kt[64:128, KC * i + b0:KC * i + b0 + nb],
                                             qsl[64:128, :], start=True, stop=True)
                            xA = expp.tile([P, 512], bf16, tag="ex", name="xA")
                            nc.scalar.activation(xA[:nb, :], eA[:nb, :], AF.Exp,
                                                 bias=nbias[:nb])
                            xB = expp.tile([P, 512], bf16, tag="ex", name="xB")
                            nc.scalar.activation(xB[:nb, :], eB[:nb, :], AF.Exp,
                                                 bias=nbias[:nb])
                            ex[0].append(xA)
                            ex[1].append(xB)
                        return ex

                    def emit_attv(s, i, ex):
                        oat = oatp.tile([P, 512], fp16, tag=f"oat{i}", name=f"oat{i}")
                        oats_by_s[s][i] = oat
                        for hl in range(2):
                            h = 2 * i + hl
                            ups = psA.tile([65, 512], f32, tag="u", name="ups", bufs=2)
                            for b, (b0, nb) in enumerate(NB):
                                nc.tensor.matmul(ups[:], vaug[b][:nb, 65 * h:65 * (h + 1)],
                                                 ex[hl][b][:nb, :],
                                                 start=(b == 0), stop=(b == 2))
                            arow = alp.tile([1, 512], f32, tag="arow")
                            nc.vector.reciprocal(arow[:], ups[64:65, :])
                            ab = alp.tile([64, 512], f32, tag="ab")
                            nc.gpsimd.partition_broadcast(ab[:], arow[:])
                            nc.vector.tensor_tensor(oat[64 * hl:64 * (hl + 1), :],
                                                    ups[0:64, :], ab[:], ALU.mult)

                    def emit_outproj(s):
                        for m in range(4):
                            fps = psA.tile([P, 512], f32, tag="f", name="fps", bufs=2)
                            for r in range(4):
                                nc.tensor.matmul(fps[:], oats_by_s[s][r][:, P * m:P * (m + 1)],
                                                 wt["wo"][:, E * r:E * (r + 1)],
                                                 start=(r == 0), stop=(r == 3))
                            ot = outp.tile([P, 512], f32, tag="ot")
                            nc.vector.tensor_copy(ot[:], fps[:])
                            nc.sync.dma_start(outd[512 * s + P * m:512 * s + P * (m + 1), :],
                                              ot[:])

                    pend = []
                    for u in units:
                        ex = emit_energy(*u)
                        pend.append((u, ex))
                        if len(pend) > 2:
                            (s_, i_), ex_ = pend.pop(0)
                            emit_attv(s_, i_, ex_)
                            if i_ == 3:
                                emit_outproj(s_)
                    for (s_, i_), ex_ in pend:
                        emit_attv(s_, i_, ex_)
                        if i_ == 3:
                            emit_outproj(s_)

    nc.compile()
    return nc


def emit_scatter(nc, sps_l, cps, oh, xhr, xlr, ch, ones_h):
    start = (ch == 0)
    stop = (ch == NCH - 1)
    for b, (b0, nb) in enumerate(NB):
        nc.tensor.matmul(sps_l[b][:nb, :], oh[:, b0:b0 + nb],
                         xhr[:, E * ch:E * (ch + 1)], start=start, stop=False)
        nc.tensor.matmul(sps_l[b][:nb, :], oh[:, b0:b0 + nb],
                         xlr[:, E * ch:E * (ch + 1)], start=False, stop=stop)
        nc.tensor.matmul(cps[b][:nb, :], oh[:, b0:b0 + nb],
                         ones_h[:], start=start, stop=stop)


_NC_CACHE = {}


def _get_nc():
    if "nc" not in _NC_CACHE:
        _NC_CACHE["nc"] = _build()
    return _NC_CACHE["nc"]


def _get_fast_runner(nc):
    """Memoized jitted executable for repeat calls.

    run_bass_kernel_spmd builds fresh closures per call, so jax re-traces and
    re-lowers every time (~1s/call). This replicates its axon execution path
    (bass2jax.run_bass_via_pjrt) once and caches the jitted callable; results
    are identical — same custom call, same NEFF.
    """
    if "runner" in _NC_CACHE:
        return _NC_CACHE["runner"]
    import jax
    from jax.sharding import Mesh, PartitionSpec
    from jax.experimental.shard_map import shard_map
    import concourse.bass2jax as b2j

    b2j.install_neuronx_cc_hook()
    partition_name = nc.partition_id_tensor.name if nc.partition_id_tensor else None
    in_names, out_names, out_avals, zero_shapes = [], [], [], []
    for alloc in nc.m.functions[0].allocations:
        if not isinstance(alloc, mybir.MemoryLocationSet):
            continue
        name = alloc.memorylocations[0].name
        if alloc.kind == "ExternalInput":
            if name != partition_name:
                in_names.append(name)
        elif alloc.kind == "ExternalOutput":
            out_names.append(name)
            shape = tuple(alloc.tensor_shape)
            dtype = mybir.dt.np(alloc.dtype)
            out_avals.append(jax.core.ShapedArray(shape, dtype))
            zero_shapes.append((shape, dtype))
    n_params = len(in_names)
    all_in = list(in_names) + list(out_names)
    if partition_name is not None:
        all_in.append(partition_name)
    donate = tuple(range(n_params, n_params + len(out_names)))

    def _body(*args):
        operands = list(args)
        if partition_name is not None:
            operands.append(b2j.partition_id_tensor())
        return tuple(b2j._bass_exec_p.bind(
            *operands, out_avals=tuple(out_avals), in_names=tuple(all_in),
            out_names=tuple(out_names), lowering_input_output_aliases=(),
            sim_require_finite=True, sim_require_nnan=True, nc=nc))

    devices = jax.devices()[:NCORES]
    mesh = Mesh(np.asarray(devices), ("core",))
    sharded = jax.jit(
        shard_map(_body, mesh=mesh,
                  in_specs=(PartitionSpec("core"),) * (n_params + len(out_names)),
                  out_specs=(PartitionSpec("core"),) * len(out_names),
                  check_rep=False),
        donate_argnums=donate, keep_unused=True)

    def run(in_maps):
        concat_in = [np.concatenate([np.asarray(m[nm]) for m in in_maps], axis=0)
                     for nm in in_names]
        concat_zeros = [np.zeros((NCORES * sh[0], *sh[1:]), dt)
                        for sh, dt in zero_shapes]
        outs = sharded(*concat_in, *concat_zeros)
        res = []
        for c in range(NCORES):
            res.append({nm: np.asarray(outs[i]).reshape(NCORES, *out_avals[i].shape)[c]
                        for i, nm in enumerate(out_names)})
        return res

    _NC_CACHE["runner"] = run
    return run


def kernel(x, A=None, Wq=None, bq=None, Wk=None, bk=None, Wv=None, bv=None,
           Wo=None, bo=None, **kw):
    x = np.asarray(x, np.float32)
    Wq = np.asarray(Wq, np.float32); Wk = np.asarray(Wk, np.float32)
    Wv = np.asarray(Wv, np.float32); Wo = np.asarray(Wo, np.float32)
    bq = np.asarray(bq, np.float32); bk = np.asarray(bk, np.float32)
    bv = np.asarray(bv, np.float32); bo = np.asarray(bo, np.float32)
    b, n, e = x.shape
    assert (b, n, e) == (1, N, E)
    x0 = x[0]
    c0 = np.ascontiguousarray(x0[INIT_IDX])
    nc = _get_nc()
    in_maps = []
    for i in range(NCORES):
        in_maps.append({
            "x": np.ascontiguousarray(x0[i * NL:(i + 1) * NL]),
            "c0": c0,
            "wq": Wq, "wk": Wk, "wv": Wv, "wo": Wo,
            "bq": bq, "bk": bk,
        })
    if "ran_once" in _NC_CACHE:
        results = _get_fast_runner(nc)(in_maps)
    else:
        res = run_bass_kernel_spmd(nc, in_maps, core_ids=list(range(NCORES)))
        results = res.results
        _NC_CACHE["ran_once"] = True
    out = np.concatenate([results[i]["out"] for i in range(NCORES)], axis=0)
    out = out + (bv @ Wo.T + bo)[None, :]
    return np.asarray(out[None], np.float32)
